# revision 24
# baseline (speedup 1.0000x reference)
"""Trainium2 Bass kernel for the E84 Neural-ODE cell.

Math (reference, dt = 1/n_steps, all elementwise on [N,N] mats per batch):
    kvqm_t = W x_t                      (projection)
    per t:  mk = m k^T, vk = v k^T
            repeat n_steps:  S += dt*(vk - sigmoid(G)*S);  G += dt*(mk - G)
            y_t = S q_t

Key restructuring used here (exact, no approximation):
  G substeps have a closed form: G_i = r^i G + (1-r^i) mk with r = 1-dt, so
  across timesteps G follows the linear scan  G_t = r^n G_{t-1} + (1-r^n) mk_t.
  With P_i = 1 - dt*sigmoid(G_i), the S update over one full timestep is
      S_t = decay_t * S_{t-1} + csum_t * dt * vk_t,
      decay_t = prod_i P_i,   csum_t = 1 + P_{n-1} + P_{n-1}P_{n-2} + ...
  Both scans run on the DVE tensor_tensor_scan (state = d0*state + d1 along
  the free dim) over an (element, time)-major layout, with a d0=0 column
  trick to reset segments at chunk boundaries. Sigmoids run on the ACT
  engine; outer products mk/vk and the readout y_t = S_t q_t run on the PE.

Layout: states are kept transposed, S'[j,i] = S[i,j], so the readout is
  y_t = S'_t^T q_t = matmul(lhsT=S'_t, rhs=q_t) directly.

Sharding: batch B=16 across 8 cores (2 per core), SPMD, no collectives.
"""

import sys

for _p in ("/opt/trn_rl_repo", "/root/.axon_site/_ro/trn_rl_repo"):
    if _p not in sys.path:
        sys.path.append(_p)

import numpy as np

import concourse.bacc as bacc
import concourse.bass as bass
import concourse.mybir as mybir
import concourse.tile as tile
from concourse.bass_utils import run_bass_kernel_spmd

FP32 = mybir.dt.float32
BF16 = mybir.dt.bfloat16
AO = mybir.AluOpType
AF = mybir.ActivationFunctionType

N = 128          # state dim
D = 1024         # model dim
NCORES = 8

_prog_cache = {}


def _build(T, B_loc, n_steps, C, prec="f32", gp_off=True):
    """Build the single-core Bass program. Returns (nc, io_names)."""
    dt = 1.0 / n_steps
    r = 1.0 - dt
    a_n = r ** n_steps                    # per-timestep G decay
    SCALE_G = (1.0 - a_n) / dt            # G_true = SCALE_G * g_state
    TB = 128                              # t-block for projection/transpose
    assert T % TB == 0 and TB % C == 0
    NTB = T // TB
    CH_PER_TB = TB // C
    FREE = N * C                          # chunk free size (i,t), t inner

    IDT = BF16 if prec == "bf16" else FP32   # intermediate dtype
    XB = 2 if prec == "bf16" else 1         # extra bufs (bf16 frees room)
    nc = bacc.Bacc("TRN2", target_bir_lowering=False, debug=False)

    x_d = nc.dram_tensor("x", [T, B_loc, D], FP32, kind="ExternalInput")
    w_d = nc.dram_tensor("w", [4 * N, D], FP32, kind="ExternalInput")
    s0_d = nc.dram_tensor("s0t", [B_loc, N, N], FP32, kind="ExternalInput")
    g0_d = nc.dram_tensor("g0ts", [B_loc, N, N], FP32, kind="ExternalInput")
    eye_d = nc.dram_tensor("eye", [N, N], FP32, kind="ExternalInput")
    y_d = nc.dram_tensor("yt", [B_loc, N, T], FP32, kind="ExternalOutput")
    sf_d = nc.dram_tensor("stf", [B_loc, N, N], FP32, kind="ExternalOutput")
    gf_d = nc.dram_tensor("gtf", [B_loc, N, N], FP32, kind="ExternalOutput")

    with tile.TileContext(nc) as tc:
        with (
            tc.tile_pool(name="const", bufs=1) as constp,
            tc.tile_pool(name="wpool", bufs=1) as wpool,
            tc.tile_pool(name="xin", bufs=2) as xin,
            tc.tile_pool(name="xtp", bufs=2) as xtp,
            tc.tile_pool(name="proj", bufs=2) as projp,
            tc.tile_pool(name="per_b", bufs=1) as perb,
            tc.tile_pool(name="chunk", bufs=1) as chp,
            tc.tile_pool(name="chain", bufs=2) as chainp,
            tc.tile_pool(name="small", bufs=2) as smallp,
            tc.tile_pool(name="psA", bufs=2, space="PSUM") as psA,
            tc.tile_pool(name="psB", bufs=4, space="PSUM") as psB,
            tc.tile_pool(name="psY", bufs=1, space="PSUM") as psY,
        ):
            # ---- constants ----
            eye = constp.tile([N, N], FP32, tag="eye")
            nc.sync.dma_start(eye[:], eye_d.ap())

            # a_n-filled d0 tile for the G scan, with zeroed t0 columns
            a4t = constp.tile([N, FREE], BF16, tag="a4t")
            nc.vector.memset(a4t[:], float(a_n))
            a4t3 = a4t[:].rearrange("p (i c) -> p i c", c=C)
            nc.vector.memset(a4t3[:, :, 0], 0.0)

            # ---- W load + transpose:  WT[dc] = [d-chunk 128, f 512] ----
            wt = [wpool.tile([N, 4 * N], FP32, name=f"wt{dc}", tag=f"wt{dc}")
                  for dc in range(8)]
            for fc in range(4):
                wn = xin.tile([N, D], FP32, tag="xt")  # reuse x-tile slots
                nc.sync.dma_start(wn[:], w_d.ap()[fc * N:(fc + 1) * N, :])
                for dc in range(8):
                    pt = psA.tile([N, TB], FP32, tag="xtp", bufs=1)
                    nc.tensor.transpose(pt[:], wn[:, dc * N:(dc + 1) * N], eye[:])
                    nc.scalar.copy(wt[dc][:, fc * N:(fc + 1) * N], pt[:])

            for b in range(B_loc):
                # per-b persistent tiles
                s0_sb = perb.tile([N, N], FP32, tag="s0_sb")
                g0_sb = perb.tile([N, N], FP32, tag="g0_sb")
                nc.sync.dma_start(s0_sb[:], s0_d.ap()[b])
                nc.sync.dma_start(g0_sb[:], g0_d.ap()[b])

                gs_prev = None   # previous chunk's G-scan output (3D view)
                ss_prev = None

                for tb in range(NTB):
                    # ---- load + transpose x block ----
                    xt = xin.tile([TB, D], FP32, tag="xt")
                    nc.sync.dma_start(xt[:], x_d.ap()[tb * TB:(tb + 1) * TB, b, :])
                    xT = []
                    for dc in range(8):
                        pt = psA.tile([N, TB], FP32, tag="xtp", bufs=1)
                        nc.tensor.transpose(pt[:], xt[:, dc * N:(dc + 1) * N], eye[:])
                        st = xtp.tile([N, TB], FP32, tag=f"xT{dc}")
                        nc.vector.tensor_copy(st[:], pt[:])
                        xT.append(st)

                    # ---- proj-T: out[t, f] for f = (k v q m) ----
                    pj = psA.tile([TB, 4 * N], FP32, tag="projT", bufs=1)
                    for dc in range(8):
                        nc.tensor.matmul(pj[:], xT[dc][:], wt[dc][:],
                                         start=(dc == 0), stop=(dc == 7))
                    # k row-space, scaled by dt (folds dt into mk and vk)
                    kT = projp.tile([TB, N], IDT, tag="kT")
                    nc.vector.tensor_scalar(kT[:], pj[:, 0:N], float(dt), None, AO.mult)
                    # rows [v | m] for the fused outer-product matmul
                    mvT = projp.tile([TB, 2 * N], IDT, tag="mvT")
                    nc.vector.tensor_copy(mvT[:, 0:N], pj[:, N:2 * N])
                    nc.scalar.copy(mvT[:, N:2 * N], pj[:, 3 * N:4 * N])

                    # ---- proj-nat: q[j, t] ----
                    pq = psA.tile([N, TB], FP32, tag="projQ", bufs=1)
                    for dc in range(8):
                        nc.tensor.matmul(pq[:], wt[dc][:, 2 * N:3 * N], xT[dc][:],
                                         start=(dc == 0), stop=(dc == 7))
                    q_tb = projp.tile([N, TB], FP32, tag="q_tb")
                    nc.scalar.copy(q_tb[:], pq[:])
                    y_tb = projp.tile([N, TB], FP32, tag="y_tb")

                    for ch in range(CH_PER_TB):
                        t0 = tb * TB + ch * C

                        # ---- granular: outer products into scan layout ----
                        # gather this chunk's k/mv rows onto partition 0 so
                        # K=1 matmuls see base_partition 0
                        kfl = smallp.tile([1, C * N], IDT, tag="kfl", bufs=XB)
                        nc.sync.dma_start(kfl[:], kT[ch * C:(ch + 1) * C, :])
                        mvfl = smallp.tile([1, C * 2 * N], IDT, tag="mvfl",
                                           bufs=XB)
                        nc.sync.dma_start(mvfl[:], mvT[ch * C:(ch + 1) * C, :])
                        mk = chp.tile([N, FREE], FP32, tag="mk", bufs=XB)
                        vk = chp.tile([N, FREE], IDT, tag="vk", bufs=XB)
                        mk3 = mk[:].rearrange("p (i c) -> p i c", c=C)
                        vk3 = vk[:].rearrange("p (i c) -> p i c", c=C)
                        for tl in range(0, C, 2):
                            # two timesteps per PSUM tile: [vk0|mk0|vk1|mk1]
                            mv = psB.tile([N, 4 * N], FP32, tag="mkvk")
                            for s2 in range(2):
                                nc.tensor.matmul(
                                    mv[:, s2 * 2 * N:(s2 + 1) * 2 * N],
                                    kfl[:, (tl + s2) * N:(tl + s2 + 1) * N],
                                    mvfl[:, (tl + s2) * 2 * N:(tl + s2 + 2) * 2 * N - 2 * N],
                                    start=True, stop=True)
                            mv4 = mv[:].rearrange("p (t x i) -> p x i t", t=2, x=2)
                            # mv4[:,0] = dt*v k^T cols, mv4[:,1] = dt*m k^T
                            nc.vector.tensor_copy(vk3[:, :, tl:tl + 2], mv4[:, 0])
                            nc.scalar.copy(mk3[:, :, tl:tl + 2], mv4[:, 1])

                        # ---- G scan ----
                        eng = nc.gpsimd if gp_off else nc.vector
                        prevG = g0_sb[:] if gs_prev is None else gs_prev[:, :, C - 1]
                        mkbak = smallp.tile([N, N], FP32, tag="mkbak")
                        eng.tensor_copy(mkbak[:], mk3[:, :, 0])
                        eng.scalar_tensor_tensor(
                            mk3[:, :, 0], prevG, float(a_n), mkbak[:],
                            AO.mult, AO.add)
                        gs = chainp.tile([N, FREE], FP32, tag="gs")
                        gs3 = gs[:].rearrange("p (i c) -> p i c", c=C)
                        nc.vector.tensor_tensor_scan(
                            gs[:], a4t[:], mk[:], 0.0, AO.mult, AO.add)

                        # ---- ladder + sigmoids (all direct from gs) ----
                        # G_i = r^i*G_0 + c_i*dt*mk, c_i = (1-r^i)/dt.
                        # Compute Gt_i = (r^i*SCALE_G/c_i)*gs + dt*mk and
                        # fold c_i into the sigmoid input scale:
                        # sg[i] = sigmoid(-c_i * Gt_i).
                        # gs[tl] is G AFTER timestep tl; the substeps of
                        # timestep tl start from gs[tl-1] (prevG at tl=0).
                        sg = []
                        s_t = chp.tile([N, FREE], IDT, tag="sig0", bufs=XB)
                        s_t3 = s_t[:].rearrange("p (i c) -> p i c", c=C)
                        nc.scalar.activation(s_t3[:, :, 1:C], gs3[:, :, 0:C - 1],
                                             AF.Sigmoid, scale=-float(SCALE_G))
                        nc.scalar.activation(s_t3[:, :, 0], prevG,
                                             AF.Sigmoid, scale=-float(SCALE_G))
                        sg.append(s_t)
                        for i in range(1, n_steps):
                            c_i = (1.0 - r ** i) / dt
                            sc = r ** i * SCALE_G / c_i
                            gl = chp.tile([N, FREE], IDT, tag=f"glad{i % 2}",
                                          name=f"glad{i}", bufs=XB)
                            gl3 = gl[:].rearrange("p (i c) -> p i c", c=C)
                            geng = nc.gpsimd if (gp_off and i >= 2) else nc.vector
                            geng.scalar_tensor_tensor(
                                gl3[:, :, 1:C], gs3[:, :, 0:C - 1],
                                float(sc), mk3[:, :, 1:C], AO.mult, AO.add)
                            eng.scalar_tensor_tensor(
                                gl3[:, :, 0], prevG, float(sc),
                                mkbak[:], AO.mult, AO.add)
                            s_t = chp.tile([N, FREE], IDT, tag=f"sig{i}",
                                           name=f"sig{i}", bufs=XB)
                            nc.scalar.activation(s_t[:], gl[:], AF.Sigmoid,
                                                 scale=-float(c_i))
                            sg.append(s_t)

                        # ---- decay (f-chain) interleaved with csum (c-chain) ----
                        # P_i = dt*(sg[i] + inv), inv = 1/dt - 1.
                        # f_k = dt^{n-1-k} * prod of (k+1) trailing Ps; f_{n-1} = decay.
                        # csum = 1 + sum_{k=0}^{n-2} f_k * dt^-(n-1-k)
                        inv = 1.0 / dt - 1.0        # 3 for dt=0.25
                        f_cur = chp.tile([N, FREE], IDT, tag="fA")
                        nc.vector.tensor_scalar(f_cur[:], sg[n_steps - 1][:],
                                                inv, float(dt ** n_steps),
                                                AO.add, AO.mult)
                        c_acc = None
                        for k in range(1, n_steps):
                            # c-term for f_{k-1}, then f_k (frees f_{k-1})
                            sc = float(dt ** -(n_steps - k))
                            if c_acc is None:
                                c_acc = chp.tile([N, FREE], IDT, tag="cA")
                                nc.vector.tensor_scalar(
                                    c_acc[:], f_cur[:], sc, None, AO.mult)
                            else:
                                c_new = chp.tile([N, FREE], IDT,
                                                 tag=f"c{'A' if k % 2 else 'B'}")
                                nc.vector.scalar_tensor_tensor(
                                    c_new[:], f_cur[:], sc, c_acc[:],
                                    AO.mult, AO.add)
                                c_acc = c_new
                            f_new = chp.tile([N, FREE], IDT,
                                             tag=f"f{'B' if k % 2 else 'A'}")
                            nc.vector.scalar_tensor_tensor(
                                f_new[:], sg[n_steps - 1 - k][:], inv, f_cur[:],
                                AO.add, AO.mult)
                            f_cur = f_new
                        decay = f_cur
                        # u = (1 + csum_partial) * vk   (vk already carries dt)
                        u = chp.tile([N, FREE], IDT, tag="u", bufs=XB)  # reuse mk slot
                        if c_acc is None:
                            nc.vector.tensor_copy(u[:], vk[:])
                        else:
                            nc.vector.scalar_tensor_tensor(
                                u[:], c_acc[:], 1.0, vk[:], AO.add, AO.mult)

                        # ---- S scan ----
                        prevS = s0_sb[:] if ss_prev is None else ss_prev[:, :, C - 1]
                        d3 = decay[:].rearrange("p (i c) -> p i c", c=C)
                        u3 = u[:].rearrange("p (i c) -> p i c", c=C)
                        stmp = smallp.tile([N, N], FP32, tag="stmp")
                        eng.tensor_tensor(stmp[:], d3[:, :, 0], prevS, AO.mult)
                        if c_acc is None:
                            nc.vector.tensor_tensor(u3[:, :, 0], stmp[:],
                                                    vk3[:, :, 0], AO.add)
                        else:
                            ufix = smallp.tile([N, N], FP32, tag="ufix")
                            c3 = c_acc[:].rearrange("p (i c) -> p i c", c=C)
                            eng.scalar_tensor_tensor(
                                ufix[:], c3[:, :, 0], 1.0, vk3[:, :, 0],
                                AO.add, AO.mult)
                            eng.tensor_tensor(u3[:, :, 0], stmp[:],
                                             ufix[:], AO.add)
                        eng.memset(d3[:, :, 0], 0.0)
                        ss = chainp.tile([N, FREE], FP32, tag="ss")
                        ss3 = ss[:].rearrange("p (i c) -> p i c", c=C)
                        nc.vector.tensor_tensor_scan(
                            ss[:], decay[:], u[:], 0.0, AO.mult, AO.add)

                        # ---- readout ----
                        yp = psY.tile([N, C], FP32, tag="yps")
                        for tl in range(C):
                            tq = ch * C + tl
                            nc.tensor.matmul(yp[:, tl:tl + 1], ss3[:, :, tl],
                                             q_tb[:, tq:tq + 1],
                                             start=True, stop=True)
                        nc.vector.tensor_copy(y_tb[:, ch * C:ch * C + C], yp[:])

                        gs_prev, ss_prev = gs3, ss3

                    nc.sync.dma_start(
                        y_d.ap()[b][:, tb * TB:(tb + 1) * TB], y_tb[:])

                # ---- final state + y out for this b ----
                gfin = smallp.tile([N, N], FP32, tag="gfin")
                nc.vector.tensor_scalar(gfin[:], gs_prev[:, :, C - 1],
                                        float(SCALE_G), None, AO.mult)
                nc.sync.dma_start(gf_d.ap()[b], gfin[:])
                sfin = smallp.tile([N, N], FP32, tag="sfin")
                nc.vector.tensor_copy(sfin[:], ss_prev[:, :, C - 1])
                nc.sync.dma_start(sf_d.ap()[b], sfin[:])

    nc.compile()
    return nc


def _get_prog(T, B_loc, n_steps, C, prec, gp_off):
    key = (T, B_loc, n_steps, C, prec, gp_off)
    if key not in _prog_cache:
        _prog_cache[key] = _build(T, B_loc, n_steps, C, prec, gp_off)
    return _prog_cache[key]


def kernel(x, W_kvqm, S0, G0, n_steps, _C=16, _sim=False,
           _prec="f32", _gp=False):
    """Full-input, full-output entry point. Shards batch across 8 cores."""
    x = np.asarray(x, dtype=np.float32)
    W = np.asarray(W_kvqm, dtype=np.float32)
    S0 = np.asarray(S0, dtype=np.float32)
    G0 = np.asarray(G0, dtype=np.float32)
    n_steps = int(n_steps)
    T, B, Dm = x.shape
    assert Dm == D and W.shape == (4 * N, D)
    ncores = min(NCORES, B)
    B_loc = B // ncores
    dt = 1.0 / n_steps
    r = 1.0 - dt
    SCALE_G = (1.0 - r ** n_steps) / dt

    nc = _get_prog(T, B_loc, n_steps, _C, _prec, _gp)

    eye = np.eye(N, dtype=np.float32)
    # device wants transposed states; G additionally prescaled by 1/SCALE_G
    S0T = np.ascontiguousarray(S0.transpose(0, 2, 1))
    G0Ts = np.ascontiguousarray(G0.transpose(0, 2, 1)) / np.float32(SCALE_G)
    in_maps = []
    for c in range(ncores):
        bs = slice(c * B_loc, (c + 1) * B_loc)
        in_maps.append({
            "x": np.ascontiguousarray(x[:, bs, :]),
            "w": W,
            "s0t": S0T[bs],
            "g0ts": np.ascontiguousarray(G0Ts[bs]).astype(np.float32),
            "eye": eye,
        })

    global last_in_maps
    last_in_maps = in_maps
    if _sim:
        from concourse.bass_interp import CoreSim
        results = []
        for c in range(ncores):
            sim = CoreSim(nc, trace=False)
            for k, v in in_maps[c].items():
                sim.tensor(k)[:] = v
            sim.simulate(check_with_hw=False)
            results.append({k: sim.tensor(k).copy()
                            for k in ("yt", "stf", "gtf")})
    else:
        res = run_bass_kernel_spmd(nc, in_maps, core_ids=list(range(ncores)))
        results = res.results
        kernel.last_results = res

    y = np.empty((T, B, N), dtype=np.float32)
    S = np.empty((B, N, N), dtype=np.float32)
    G = np.empty((B, N, N), dtype=np.float32)
    for c in range(ncores):
        for bl in range(B_loc):
            b = c * B_loc + bl
            y[:, b, :] = results[c]["yt"][bl].T
            S[b] = results[c]["stf"][bl].T
            G[b] = results[c]["gtf"][bl].T
    return y, S, G
